# revision 13
# baseline (speedup 1.0000x reference)
"""Trainium2 Bass kernel for nn_MultiLevelPooling (segment_reduce).

Strategy (8 NeuronCores, SPMD):
  - `batch` is sorted, so graph g's nodes are a contiguous node range
    (found host-side with searchsorted). Core c owns graphs
    [128c, 128(c+1)) -> a contiguous slice of nodes. No collectives.
  - Per core, two bf16 layouts of its node slice are staged host-side:
      * natural [node, feat] tiles -> PE one-hot matmul computes the
        segment SUM (exact f32 PSUM accumulation),
      * transposed [feat, node] with per-segment padding to a shared
        (max-over-cores) length profile -> DVE tensor_tensor_reduce
        computes the segment MAX per segment column range.
  - Counts come free from searchsorted boundaries; 1/max(count,1) is
    shipped as a tiny broadcast tile.
  - The downstream dense net (3 transforms + gated softmax fusion +
    out-proj + layernorm) runs per-core on its 128 graphs.
  - Host concatenates the 8 per-core [128, 256] outputs.
"""

import os
import sys

for _p in ("/opt/trn_rl_repo", "/root/.axon_site/_ro/trn_rl_repo"):
    if os.path.isdir(_p) and _p not in sys.path:
        sys.path.insert(0, _p)

from contextlib import ExitStack

import ml_dtypes
import numpy as np

from concourse import bacc, bass, bass_utils, mybir, tile
from concourse.bass_interp import get_hw_module

BF16 = ml_dtypes.bfloat16

G = 1024  # num graphs (segments)
F = 256  # in features
H = 512  # hidden
NCORES = 8
GPC = G // NCORES  # graphs per core = 128
P = 128  # partitions
FH = F // P  # feature halves = 2
HT = H // P  # hidden tiles = 4

TILE_L = 2048  # xT tile free length (columns)
G_NAT = 8  # natural-layout node tiles per DMA group
NEG_INF = -3.0e38

Alu = mybir.AluOpType
Act = mybir.ActivationFunctionType
DT = mybir.dt


# ---------------------------------------------------------------------------
# Host-side prep
# ---------------------------------------------------------------------------

def _host_prep(x, batch):
    """Compute shared layout meta + per-core staged arrays."""
    N = x.shape[0]
    batch = np.asarray(batch).astype(np.int64)
    if not np.all(batch[1:] >= batch[:-1]):
        order = np.argsort(batch, kind="stable")
        batch = batch[order]
        x = np.asarray(x)[order]

    starts = np.searchsorted(batch, np.arange(G), side="left")
    ends = np.searchsorted(batch, np.arange(G), side="right")
    counts = (ends - starts).astype(np.int64)  # [G]

    # Per-core node ranges
    core_lo = np.array([starts[c * GPC] for c in range(NCORES)])
    core_hi = np.array([ends[(c + 1) * GPC - 1] for c in range(NCORES)])
    nodes_per_core = core_hi - core_lo
    maxn = int(nodes_per_core.max())
    NT = max(1, -(-maxn // P))  # node tiles per core
    NTG = -(-NT // G_NAT)  # DMA groups
    NT = NTG * G_NAT

    # Per-position padded segment lengths: PAD_k = max-over-cores count at
    # local position k, rounded up to a multiple of 16. Positions are
    # reordered (shared permutation) so equal-PAD segments are contiguous,
    # giving each bucket a uniform fold-tree structure on every core.
    cnt_mat = counts.reshape(NCORES, GPC)  # [core, k]
    lam = cnt_mat.max(axis=0)
    pads = np.maximum(16, -(-lam // 16) * 16).astype(np.int64)  # [GPC]
    perm = np.argsort(pads, kind="stable")  # device col j <- local seg perm[j]
    pads_p = pads[perm]
    col_off = np.zeros(GPC + 1, np.int64)
    col_off[1:] = np.cumsum(pads_p)
    NPAD = int(col_off[-1])
    rank = np.empty(GPC, np.int64)
    rank[perm] = np.arange(GPC)
    # bucket runs: (j0, nsegs, pad)
    buckets = []
    j = 0
    while j < GPC:
        j2 = j
        while j2 < GPC and pads_p[j2] == pads_p[j]:
            j2 += 1
        buckets.append((int(j), int(j2 - j), int(pads_p[j])))
        j = j2

    x_bf = np.asarray(x, np.float32).astype(BF16)
    # extended with one zero row for padding gathers
    x_ext = np.concatenate([x_bf, np.zeros((1, F), BF16)], axis=0)

    meta = dict(NT=NT, NTG=NTG, buckets=tuple(buckets),
                col_off0=tuple(int(v) for v in col_off[:-1]))

    iota_row = np.tile(np.arange(P, dtype=np.float32), (P, 1)).astype(BF16)

    in_maps = []
    for c in range(NCORES):
        lo, hi = int(core_lo[c]), int(core_hi[c])
        n_c = hi - lo
        # natural layout [NT*P, F] bf16 (pad rows -> zero row)
        nat_idx = np.full(NT * P, N, np.int64)
        nat_idx[:n_c] = np.arange(lo, hi)
        x_nat = np.ascontiguousarray(x_ext[nat_idx])  # [NT*P, F]
        # local seg ids per node tile-major [P, NT] bf16 (255 for pads)
        bl = np.full(NT * P, 255, np.int64)
        bl[:n_c] = rank[batch[lo:hi] - c * GPC]
        bcols = np.ascontiguousarray(
            bl.reshape(NT, P).T.astype(np.float32))
        # transposed padded layout [F, NPAD], device col block j holds
        # local segment perm[j] padded to pads_p[j]
        t_idx = np.full(NPAD, N, np.int64)
        for j in range(GPC):
            g = c * GPC + int(perm[j])
            cnt = int(counts[g])
            o = int(col_off[j])
            if cnt > 0:
                t_idx[o:o + cnt] = np.arange(starts[g], ends[g])
                t_idx[o + cnt:o + int(pads_p[j])] = ends[g] - 1
            # cnt == 0 -> stays N (zero column) => max = 0 like reference
        xT = np.ascontiguousarray(x_ext[t_idx].T)  # [F, NPAD] bf16
        # 1/max(count,1) broadcast [P, GPC] f32
        rmean = (1.0 / np.maximum(
            counts[c * GPC:(c + 1) * GPC][perm], 1)).astype(np.float32)
        rmean_b = np.ascontiguousarray(np.tile(rmean, (P, 1)))
        in_maps.append(dict(
            x_nat=x_nat, bcols=bcols, xT=xT, rmean=rmean_b,
            iota=iota_row, ident=np.eye(P, dtype=np.float32),
        ))
    meta["perm"] = tuple(int(v) for v in perm)
    return meta, in_maps


def _prep_weights(W_mean, b_mean, W_max, b_max, W_sum, b_sum,
                  g_mean_w, g_mean_b, g_max_w, g_max_b, g_sum_w, g_sum_b,
                  W_out, b_out, ln_gamma, ln_beta):
    """Weight arrays (replicated to every core) + scalar immediates."""
    def bf(a):
        return np.ascontiguousarray(np.asarray(a, np.float32).astype(BF16))

    def f32(a):
        return np.ascontiguousarray(np.asarray(a, np.float32))

    wmaps = dict(
        Wm=f32(W_mean), Wx=f32(W_max), Ws=f32(W_sum),
        # biases [H] -> [P, HT] (column ht = partitions of h-tile ht)
        bm=f32(np.reshape(b_mean, (HT, P)).T),
        bx=f32(np.reshape(b_max, (HT, P)).T),
        bs=f32(np.reshape(b_sum, (HT, P)).T),
        gw=f32(np.concatenate(
            [np.reshape(g_mean_w, (H, 1)), np.reshape(g_max_w, (H, 1)),
             np.reshape(g_sum_w, (H, 1))], axis=1)),  # [H, 3]
        Wout=f32(W_out),  # [H, F]
        bout=f32(np.tile(np.reshape(b_out, (1, F)), (P, 1))),
        gamma=f32(np.tile(np.reshape(ln_gamma, (1, F)), (P, 1))),
        beta=f32(np.tile(np.reshape(ln_beta, (1, F)), (P, 1))),
    )
    scalars = dict(
        gb=(float(np.reshape(g_mean_b, (-1,))[0]),
            float(np.reshape(g_max_b, (-1,))[0]),
            float(np.reshape(g_sum_b, (-1,))[0])),
    )
    return wmaps, scalars


# ---------------------------------------------------------------------------
# Device program
# ---------------------------------------------------------------------------

def _build_body(ctx, tc, d, meta, scalars):
    """Emit one iteration of the per-core compute. `d` maps name->dram AP."""
    nc = tc.nc
    NT, NTG = meta["NT"], meta["NTG"]

    const = ctx.enter_context(tc.tile_pool(name="const", bufs=1))
    io = ctx.enter_context(tc.tile_pool(name="io", bufs=3))
    stats = ctx.enter_context(tc.tile_pool(name="stats", bufs=1))

    # --- constants / small inputs ---
    iota_sb = const.tile([P, P], DT.bfloat16, tag="iota")
    nc.sync.dma_start(iota_sb[:], d["iota"][:])
    bcols_sb = const.tile([P, NT], DT.float32, tag="bcols")
    nc.sync.dma_start(bcols_sb[:], d["bcols"][:])
    rmean_sb = const.tile([P, GPC], DT.float32, tag="rmean")
    nc.sync.dma_start(rmean_sb[:], d["rmean"][:])



    # --- segment max via TT-max fold tree over padded/bucketed xT ---
    # Device column block j holds one segment padded to its bucket's PAD.
    # Per bucket: view tiles as [P, SEGT, PAD], halve the innermost axis
    # with TT max (2x bf16) until width <= 8 or odd, then one 3D-AP
    # tensor_reduce writes the per-segment maxes.
    buckets = meta["buckets"]
    col_off0 = meta["col_off0"]
    maxT_sb = [stats.tile([P, GPC], DT.float32, tag=f"maxT{fh}", bufs=2,
                          name=f"maxT{fh}")
               for fh in range(FH)]
    for fh in range(FH):
        for (j0, nseg_b, PAD) in buckets:
            SEGT = max(1, min(nseg_b, 4096 // PAD))
            NXT = -(-nseg_b // SEGT)
            base = col_off0[j0]
            xv = d["xT"][fh * P:(fh + 1) * P,
                         base:base + nseg_b * PAD].rearrange(
                "f (k q) -> f k q", q=PAD)
            for it in range(NXT):
                k0 = it * SEGT
                ns = min(SEGT, nseg_b - k0)
                xt = io.tile([P, 4096], DT.bfloat16, tag="xt", bufs=5)
                xtv = xt[:, :SEGT * PAD].rearrange("f (k q) -> f k q", q=PAD)
                nc.sync.dma_start(xtv[:, :ns, :], xv[:, k0:k0 + ns, :])
                cur, cur_w = xtv, PAD
                si = 0
                while cur_w > 8 and cur_w % 2 == 0:
                    nw = cur_w // 2
                    scr = io.tile([P, 2048], DT.bfloat16,
                                  tag=f"scr{si}", bufs=2, name=f"scr{si}")
                    scrv = scr[:, :SEGT * nw].rearrange(
                        "f (k q) -> f k q", q=nw)
                    nc.vector.tensor_tensor(
                        out=scrv[:, :ns, :], in0=cur[:, :ns, :nw],
                        in1=cur[:, :ns, nw:cur_w], op=Alu.max)
                    cur, cur_w = scrv, nw
                    si += 1
                nc.vector.tensor_reduce(
                    out=maxT_sb[fh][:, j0 + k0:j0 + k0 + ns],
                    in_=cur[:, :ns, :cur_w],
                    axis=mybir.AxisListType.X, op=Alu.max)

    # --- segment sum via one-hot matmul: lhsT=onehot (one ldweights per
    # node tile), rhs=x natural tile [P, F] -> PSUM accum [GPC, F] natural.
    # Then PE-transpose the two feature halves to get sum_poolT [f, g].
    with tc.tile_pool(name="psum_sum", bufs=1,
                      space=bass.MemorySpace.PSUM) as psum_sum:
        sum_ps = psum_sum.tile([P, F], DT.float32, tag="sum", name="sumps")
        nat_view = d["x_nat"].rearrange("(t p) f -> p t f", p=P)
        for tg in range(NTG):
            xg = io.tile([P, G_NAT, F], DT.bfloat16, tag="xg", bufs=6)
            nc.sync.dma_start(
                xg[:], nat_view[:, tg * G_NAT:(tg + 1) * G_NAT, :])
            for j in range(G_NAT):
                t = tg * G_NAT + j
                oh = io.tile([P, P], DT.bfloat16, tag="oh", bufs=8)
                nc.gpsimd.tensor_scalar(
                    out=oh[:], in0=iota_sb[:],
                    scalar1=bcols_sb[:, t:t + 1], scalar2=None,
                    op0=Alu.is_equal)
                nc.tensor.matmul(
                    sum_ps[:], oh[:], xg[:, j, :],
                    start=(t == 0), stop=(t == NT - 1))
        sum_nat = stats.tile([P, F], DT.float32, tag="sum_nat")
        nc.scalar.copy(sum_nat[:], sum_ps[:])
    # --- weights / downstream constants (loaded after the big streams) ---
    ident_sb = const.tile([P, P], DT.float32, tag="ident")
    nc.sync.dma_start(ident_sb[:], d["ident"][:])
    Wsb = {}
    for nm in ("Wm", "Wx", "Ws"):
        t = const.tile([P, FH, H], DT.float32, tag=nm, name=nm)
        nc.sync.dma_start(t[:], d[nm].rearrange("(kt p) h -> p kt h", p=P))
        Wsb[nm] = t
    bsb = {}
    for nm in ("bm", "bx", "bs"):
        t = const.tile([P, HT], DT.float32, tag=nm, name=nm)
        nc.sync.dma_start(t[:], d[nm][:])
        bsb[nm] = t
    gw_sb = const.tile([P, HT, 3], DT.float32, tag="gw")
    nc.sync.dma_start(gw_sb[:], d["gw"].rearrange("(kt p) g -> p kt g", p=P))
    wout_sb = const.tile([P, HT, F], DT.float32, tag="wout")
    nc.sync.dma_start(wout_sb[:], d["Wout"].rearrange("(ht p) f -> p ht f", p=P))
    bout_sb = const.tile([P, F], DT.float32, tag="bout")
    nc.sync.dma_start(bout_sb[:], d["bout"][:])
    gamma_sb = const.tile([P, F], DT.float32, tag="gamma")
    nc.sync.dma_start(gamma_sb[:], d["gamma"][:])
    beta_sb = const.tile([P, F], DT.float32, tag="beta")
    nc.sync.dma_start(beta_sb[:], d["beta"][:])
    ones_row = const.tile([1, P], DT.float32, tag="ones_row")
    nc.vector.memset(ones_row[:], 1.0)

    sumT_bf = [stats.tile([P, GPC], DT.float32, tag=f"sumbf{fh}", name=f"sumbf{fh}")
               for fh in range(FH)]
    meanT_bf = [stats.tile([P, GPC], DT.float32, tag=f"meanbf{fh}", name=f"meanbf{fh}")
                for fh in range(FH)]
    maxT_bf = maxT_sb
    with tc.tile_pool(name="psum_tr", bufs=2,
                      space=bass.MemorySpace.PSUM) as psum_tr:
        for fh in range(FH):
            trp = psum_tr.tile([P, P], DT.float32, tag="trp", bufs=2)
            nc.tensor.transpose(
                trp[:], sum_nat[:, fh * P:(fh + 1) * P], ident_sb[:])
            nc.scalar.copy(sumT_bf[fh][:], trp[:])
            nc.vector.tensor_tensor(
                out=meanT_bf[fh][:], in0=trp[:], in1=rmean_sb[:],
                op=Alu.mult)

    # --- transforms: reprT[h, g] = W^T @ poolT (+bias) for mean/max/sum
    reprs = {}
    with tc.tile_pool(name="psum_repr", bufs=4,
                      space=bass.MemorySpace.PSUM) as psum_repr:
        for nm, wname, bname, poolT in (
                ("mean", "Wm", "bm", meanT_bf),
                ("max", "Wx", "bx", maxT_bf),
                ("sum", "Ws", "bs", sumT_bf)):
            rsb = stats.tile([P, HT, GPC], DT.float32, tag=f"repr_{nm}")
            for ht in range(HT):
                rp = psum_repr.tile([P, GPC], DT.float32, tag="rp")
                for kt in range(FH):
                    nc.tensor.matmul(
                        rp[:], Wsb[wname][:, kt, ht * P:(ht + 1) * P],
                        poolT[kt][:],
                        start=(kt == 0), stop=(kt == FH - 1))
                # +bias (per-partition) and evacuate to bf16 sbuf
                nc.scalar.activation(
                    rsb[:, ht, :], rp[:], Act.Identity,
                    bias=bsb[bname][:, ht:ht + 1], scale=1.0)
            reprs[nm] = rsb

    # --- gates ---
    gate_w_bc = []
    with tc.tile_pool(name="psum_gate", bufs=3,
                      space=bass.MemorySpace.PSUM) as psum_gate, \
            tc.tile_pool(name="psum_bc", bufs=3,
                         space=bass.MemorySpace.PSUM) as psum_bc, \
            tc.tile_pool(name="gates", bufs=1) as gpool:
        eg = []
        for gi, nm in enumerate(("mean", "max", "sum")):
            gp = psum_gate.tile([1, GPC], DT.float32, tag="gp", bufs=3)
            for kt in range(HT):
                nc.tensor.matmul(
                    gp[:], gw_sb[:, kt, gi:gi + 1], reprs[nm][:, kt, :],
                    start=(kt == 0), stop=(kt == HT - 1))
            gb_ap = gpool.tile([1, 1], DT.float32, tag=f"gb{gi}",
                               name=f"gb{gi}")
            nc.vector.memset(gb_ap[:], float(scalars["gb"][gi]))
            sg = gpool.tile([1, GPC], DT.float32, tag=f"sg{gi}")
            nc.scalar.activation(sg[:], gp[:], Act.Sigmoid,
                                 bias=gb_ap[:], scale=1.0)
            e = gpool.tile([1, GPC], DT.float32, tag=f"e{gi}")
            nc.scalar.activation(e[:], sg[:], Act.Exp)
            eg.append(e)
        esum = gpool.tile([1, GPC], DT.float32, tag="esum")
        nc.vector.tensor_tensor(out=esum[:], in0=eg[0][:], in1=eg[1][:],
                                op=Alu.add)
        nc.vector.tensor_tensor(out=esum[:], in0=esum[:], in1=eg[2][:],
                                op=Alu.add)
        erec = gpool.tile([1, GPC], DT.float32, tag="erec")
        nc.vector.reciprocal(erec[:], esum[:])
        for gi in range(3):
            wbf = gpool.tile([1, GPC], DT.float32, tag=f"wbf{gi}")
            nc.vector.tensor_tensor(out=wbf[:], in0=eg[gi][:], in1=erec[:],
                                    op=Alu.mult)
            bc = psum_bc.tile([P, GPC], DT.float32, tag="bc", bufs=3)
            nc.tensor.matmul(bc[:], ones_row[:], wbf[:])
            wb = gpool.tile([P, GPC], DT.float32, tag=f"wb{gi}")
            nc.vector.tensor_copy(wb[:], bc[:])
            gate_w_bc.append(wb)

        # --- gated combine: pooledT[ht] = sum_i w_i * repr_i[ht]
        pooledT = stats.tile([P, HT, GPC], DT.float32, tag="pooledT")
        acc = gpool.tile([P, GPC], DT.float32, tag="acc", bufs=2)
        for ht in range(HT):
            nc.vector.tensor_tensor(
                out=acc[:], in0=reprs["mean"][:, ht, :], in1=gate_w_bc[0][:],
                op=Alu.mult)
            t2 = gpool.tile([P, GPC], DT.float32, tag="t2", bufs=2)
            nc.vector.tensor_tensor(
                out=t2[:], in0=reprs["max"][:, ht, :], in1=gate_w_bc[1][:],
                op=Alu.mult)
            nc.vector.tensor_tensor(out=acc[:], in0=acc[:], in1=t2[:],
                                    op=Alu.add)
            nc.vector.tensor_tensor(
                out=t2[:], in0=reprs["sum"][:, ht, :], in1=gate_w_bc[2][:],
                op=Alu.mult)
            nc.vector.tensor_tensor(out=pooledT[:, ht, :], in0=acc[:],
                                    in1=t2[:], op=Alu.add)

        # --- output projection + layernorm ---
        with tc.tile_pool(name="psum_emb", bufs=1,
                          space=bass.MemorySpace.PSUM) as psum_emb:
            ep = psum_emb.tile([P, F], DT.float32, tag="ep")
            for ht in range(HT):
                nc.tensor.matmul(ep[:], pooledT[:, ht, :], wout_sb[:, ht, :],
                                 start=(ht == 0), stop=(ht == HT - 1))
            emb = gpool.tile([P, F], DT.float32, tag="emb")
            nc.vector.tensor_tensor(out=emb[:], in0=ep[:], in1=bout_sb[:],
                                    op=Alu.add)
        bnst = gpool.tile([P, 6], DT.float32, tag="bnst")
        nc.vector.bn_stats(bnst[:], emb[:])
        bnag = gpool.tile([P, 2], DT.float32, tag="bnag")
        nc.vector.bn_aggr(bnag[:], bnst[:])
        mu = bnag[:, 0:1]
        var = bnag[:, 1:2]
        tv = gpool.tile([P, 1], DT.float32, tag="tv")
        nc.vector.tensor_scalar_add(tv[:], var, 1e-5)
        rv = gpool.tile([P, 1], DT.float32, tag="rv")
        nc.vector.reciprocal(rv[:], tv[:])
        rs0 = gpool.tile([P, 1], DT.float32, tag="rs0")
        nc.scalar.sqrt(rs0[:], rv[:])
        # one Newton step: rs = rs0 * (1.5 - 0.5 * tv * rs0^2)
        t1 = gpool.tile([P, 1], DT.float32, tag="t1")
        nc.vector.tensor_tensor(out=t1[:], in0=rs0[:], in1=rs0[:],
                                op=Alu.mult)
        nc.vector.tensor_tensor(out=t1[:], in0=t1[:], in1=tv[:], op=Alu.mult)
        nc.vector.tensor_scalar(out=t1[:], in0=t1[:], scalar1=-0.5,
                                scalar2=1.5, op0=Alu.mult, op1=Alu.add)
        rs = gpool.tile([P, 1], DT.float32, tag="rs")
        nc.vector.tensor_tensor(out=rs[:], in0=rs0[:], in1=t1[:],
                                op=Alu.mult)
        nmurs = gpool.tile([P, 1], DT.float32, tag="nmurs")
        nc.vector.tensor_tensor(out=nmurs[:], in0=mu, in1=rs[:], op=Alu.mult)
        nc.vector.tensor_scalar_mul(nmurs[:], nmurs[:], -1.0)
        e1 = gpool.tile([P, F], DT.float32, tag="e1")
        nc.scalar.activation(e1[:], emb[:], Act.Identity,
                             bias=nmurs[:], scale=rs[:])
        e2 = gpool.tile([P, F], DT.float32, tag="e2")
        nc.vector.tensor_tensor(out=e2[:], in0=e1[:], in1=gamma_sb[:],
                                op=Alu.mult)
        nc.vector.tensor_tensor(out=e2[:], in0=e2[:], in1=beta_sb[:],
                                op=Alu.add)
        nc.sync.dma_start(d["y"][:], e2[:])


def _build_program(meta, scalars, wshapes, in_shapes, reps=1, hw=True):
    nc = bacc.Bacc("TRN2", target_bir_lowering=False, debug=False,
                   num_devices=NCORES)
    d = {}
    for nm, (shape, np_dt) in in_shapes.items():
        bdt = DT.from_np(np.dtype(np_dt))
        d[nm] = nc.dram_tensor(nm, list(shape), bdt,
                               kind="ExternalInput").ap()
    d["y"] = nc.dram_tensor("y", [P, F], DT.float32,
                            kind="ExternalOutput").ap()
    with tile.TileContext(nc, trace_sim=False) as tc:
        for _ in range(reps):
            with ExitStack() as ctx:
                _build_body(ctx, tc, d, meta, scalars)
    nc.compile()
    if hw:
        nc.m = get_hw_module(nc.m)
    return nc


_CACHE = {}


def _get_program(meta, scalars, in_maps, wmaps, reps=1):
    shapes = {}
    for nm, a in in_maps[0].items():
        shapes[nm] = (a.shape, a.dtype)
    for nm, a in wmaps.items():
        shapes[nm] = (a.shape, a.dtype)
    key = (repr(sorted((k, v[0], str(v[1])) for k, v in shapes.items())),
           repr(meta), repr(scalars), reps)
    if key not in _CACHE:
        _CACHE[key] = _build_program(meta, scalars, wmaps, shapes, reps=reps)
    return _CACHE[key]


def kernel(x, batch, W_mean, b_mean, W_max, b_max, W_sum, b_sum,
           g_mean_w, g_mean_b, g_max_w, g_max_b, g_sum_w, g_sum_b,
           W_out, b_out, ln_gamma, ln_beta, _reps=1, _return_res=False):
    x = np.asarray(x, np.float32)
    meta, in_maps = _host_prep(x, batch)
    wmaps, scalars = _prep_weights(
        W_mean, b_mean, W_max, b_max, W_sum, b_sum,
        g_mean_w, g_mean_b, g_max_w, g_max_b, g_sum_w, g_sum_b,
        W_out, b_out, ln_gamma, ln_beta)
    for m in in_maps:
        m.update(wmaps)
    nc = _get_program(meta, scalars, in_maps, wmaps, reps=_reps)
    res = bass_utils.run_bass_kernel_spmd(
        nc, in_maps, core_ids=list(range(NCORES)))
    out = np.concatenate([res.results[c]["y"] for c in range(NCORES)],
                         axis=0).astype(np.float32)
    if _return_res:
        return out, res
    return out
